# revision 7
# baseline (speedup 1.0000x reference)
"""VQ codebook kernel for 8 Trainium2 NeuronCores (Bass/Tile).

For z [16, 4096, 64] f32 and emb [1024, 64] f32, computes (matching the
reference bit-for-bit on this machine):
  idx  = argmin_k ||z - e_k||^2   (fp32, XLA op order)    [16, 4096] int32
  z_q  = z + (emb[idx] - z)                               [16, 4096, 64] f32
  loss = sum_b (0.25*mean_b + mean_b), mean_b = mean((emb[idx]-z)^2)

Sharding: data-parallel; core c owns tokens [c*8192, (c+1)*8192) = batches
2c, 2c+1. Codebook replicated; loss partials combined on host.

Device pipeline per core (64 tiles of 128 tokens):
- PE: one fp32 matmul per 512-col half computes w = tree(2 z.e - B) straight
  into PSUM (zT carries an all-ones row 64; embT2 carries -B in row 64).
  The PE fp32 matmul is bit-identical to XLA-neuron's, so w reproduces the
  reference's distance ordering except at its coarse-grid rounding ties.
- DVE: max (top-8) + max_index per tile -> argmax of w = argmin of dist,
  first-occurrence tie-break; also diff = q - z and z_q = z + diff.
- GPSIMD: ap_gather ucode gathers q[d, t] = embT[d, idx[t]] (all 8 Q7 cores;
  batch-halves stacked across partitions 0-63 / 64-127).
- ACT: Square+accumulate for the loss partial sums.

Host: input shard/transpose prep, and a repair pass - tokens whose top-2 w
gap is within ~6 ulps of the reference's fl(fl(A+B)-2M) grid (~0.5%) get an
exact recompute, which also covers max_index tie semantics. A ulp-shift
argument shows A cannot otherwise affect the argmin, so it lives host-side
only.
"""

import sys

for _p in ("/opt/trn_rl_repo", "/root/.axon_site/_ro/trn_rl_repo"):
    if _p not in sys.path:
        sys.path.insert(0, _p)

import numpy as np

import concourse.bacc as bacc
import concourse.mybir as mybir
from concourse import library_config
from concourse.tile import TileContext
from concourse.bass_utils import run_bass_kernel_spmd

B, T, D, K = 16, 4096, 64, 1024
N_CORES = 8
TOK = (B * T) // N_CORES          # 8192 tokens per core
HT = TOK // 2                     # 4096 tokens per batch-half
N_TILES = TOK // 128              # 64 tiles
N_BATCH = 4                       # gather batches; each covers 1024 tokens/half
BT = HT // N_BATCH                # 1024 tokens per half per batch
TPB = BT // 128                   # 8 tiles per half per batch

F32 = mybir.dt.float32
U16 = mybir.dt.uint16
I16 = mybir.dt.int16

_cached = {}


def _build_program():
    nc = bacc.Bacc(None, target_bir_lowering=False, debug=False)

    zT_d = nc.dram_tensor("zT", [D + 1, TOK], F32, kind="ExternalInput")
    embT2_d = nc.dram_tensor("embT2", [D + 1, K], F32, kind="ExternalInput")
    embT1_d = nc.dram_tensor("embT1", [128, K], F32, kind="ExternalInput")
    zS_d = nc.dram_tensor("zS", [128, HT], F32, kind="ExternalInput")

    zqS_d = nc.dram_tensor("zqS", [128, HT], F32, kind="ExternalOutput")
    idxu_d = nc.dram_tensor("idxu", [128, N_TILES], U16, kind="ExternalOutput")
    maxes_d = nc.dram_tensor("maxes", [128, N_TILES, 2], F32, kind="ExternalOutput")
    losspart_d = nc.dram_tensor("losspart", [128, 2 * N_BATCH], F32, kind="ExternalOutput")

    with TileContext(nc) as tc:
        with (
            tc.tile_pool(name="singles", bufs=1) as singles,
            tc.tile_pool(name="dfp", bufs=2) as dfp,
            tc.tile_pool(name="zqp", bufs=2) as zqp,
            tc.tile_pool(name="sqp", bufs=2) as sqp,
            tc.tile_pool(name="pswp", bufs=3, space="PSUM") as pswp,
        ):
            zT_sb = singles.tile([D + 1, TOK], F32)
            embT2_sb = singles.tile([D + 1, K], F32)
            embT1_sb = singles.tile([128, K, 1], F32)
            zS_sb = singles.tile([128, HT], F32)
            max8_all = singles.tile([128, N_TILES, 8], F32)
            idx8_all = singles.tile([128, N_TILES, 8], U16)
            idxw = singles.tile([128, HT // 16], I16)
            qS_all = singles.tile([128, HT, 1], F32)
            losspart_sb = singles.tile([128, 2 * N_BATCH], F32)

            nc.sync.dma_start(out=embT2_sb, in_=embT2_d[:, :])
            for ch in range(8):
                cs = slice(ch * (TOK // 8), (ch + 1) * (TOK // 8))
                nc.sync.dma_start(out=zT_sb[:, cs], in_=zT_d[:, cs])
            nc.sync.dma_start(out=embT1_sb[:, :, 0], in_=embT1_d[:, :])
            for ch in range(4):
                cs = slice(ch * (HT // 4), (ch + 1) * (HT // 4))
                nc.sync.dma_start(out=zS_sb[:, cs], in_=zS_d[:, cs])

            nc.gpsimd.load_library(library_config.ap_gather)

            for b in range(N_BATCH):
                for half in range(2):
                    for j in range(TPB):
                        g = half * (N_TILES // 2) + b * TPB + j
                        tok = slice(g * 128, (g + 1) * 128)
                        psw = pswp.tile([128, K], F32)
                        for h in range(2):
                            hs = slice(h * 512, (h + 1) * 512)
                            nc.tensor.matmul(
                                psw[:, hs], lhsT=zT_sb[:, tok], rhs=embT2_sb[:, hs],
                                start=True, stop=True,
                            )
                        nc.vector.max(out=max8_all[:, g, :], in_=psw)
                        nc.vector.max_index(idx8_all[:, g, :], max8_all[:, g, :], psw)

                # Wrapped idx lists: token i of batch b / half h sits at
                # (partition i%16 + 64*h [+16*rep], col 64*b + i//16).
                # Source token i = tile (64/2)*h + b*8 + i//128, p = i%128.
                bcol = slice(b * (BT // 16), (b + 1) * (BT // 16))
                for half in range(2):
                    g0 = half * (N_TILES // 2) + b * TPB
                    pbase = 64 * half
                    for r in range(8):
                        nc.sync.dma_start(
                            out=idxw[pbase : pbase + 16,
                                     b * (BT // 16) + r : (b + 1) * (BT // 16) : 8],
                            in_=idx8_all[16 * r : 16 * (r + 1), g0 : g0 + TPB, 0].bitcast(I16),
                        )
                    for rep in range(1, 4):
                        nc.sync.dma_start(
                            out=idxw[pbase + 16 * rep : pbase + 16 * (rep + 1), bcol],
                            in_=idxw[pbase : pbase + 16, bcol],
                        )

                nc.gpsimd.ap_gather(
                    qS_all[:, b * BT : (b + 1) * BT, :],
                    embT1_sb,
                    idxw[:, bcol],
                    channels=128,
                    num_elems=K,
                    d=1,
                    num_idxs=BT,
                )

                for gg in range(2):
                    g_out = 2 * b + gg
                    gtok = slice(g_out * 512, (g_out + 1) * 512)
                    diff = dfp.tile([128, 512], F32)
                    nc.vector.tensor_tensor(
                        diff, qS_all[:, gtok, 0], zS_sb[:, gtok],
                        op=mybir.AluOpType.subtract,
                    )
                    zq = zqp.tile([128, 512], F32)
                    nc.vector.tensor_tensor(
                        zq, zS_sb[:, gtok], diff, op=mybir.AluOpType.add
                    )
                    sq = sqp.tile([128, 512], F32)
                    nc.scalar.activation(
                        sq, diff, mybir.ActivationFunctionType.Square,
                        accum_out=losspart_sb[:, g_out : g_out + 1],
                    )
                    nc.sync.dma_start(out=zqS_d[:, gtok], in_=zq)

            nc.sync.dma_start(out=idxu_d[:, :], in_=idx8_all[:, :, 0])
            nc.sync.dma_start(out=maxes_d[:, :, :], in_=max8_all[:, :, 0:2])
            nc.sync.dma_start(out=losspart_d[:, :], in_=losspart_sb)

    nc.compile()
    return nc


def _get_program():
    if "nc" not in _cached:
        _cached["nc"] = _build_program()
    return _cached["nc"]


def kernel(z, emb):
    z = np.asarray(z, dtype=np.float32)
    emb = np.asarray(emb, dtype=np.float32)
    zf = z.reshape(-1, D)

    A = (zf * zf).sum(axis=1).astype(np.float32)
    Bv = (emb * emb).sum(axis=1).astype(np.float32)
    embT2 = np.empty((D + 1, K), np.float32)
    embT2[:D] = (emb * np.float32(2.0)).T
    embT2[D] = -Bv
    embT1 = np.empty((128, K), np.float32)
    embT1[:D] = emb.T
    embT1[D:] = emb.T

    in_maps = []
    for c in range(N_CORES):
        zc = zf[c * TOK : (c + 1) * TOK]
        z65 = np.empty((D + 1, TOK), np.float32)
        z65[:D] = zc.T
        z65[D] = 1.0
        zS = np.empty((128, HT), np.float32)
        zS[:D] = zc[:HT].T
        zS[D:] = zc[HT:].T
        in_maps.append({"zT": z65, "embT2": embT2, "embT1": embT1, "zS": zS})

    nc = _get_program()
    res = run_bass_kernel_spmd(
        nc, in_maps, core_ids=list(range(N_CORES)), **_cached.get("run_kwargs", {})
    )
    _cached["last_res"] = res

    z_q = np.empty((B * T, D), dtype=np.float32)
    idx = np.empty(B * T, dtype=np.int32)
    S = np.zeros(B, dtype=np.float64)

    for c in range(N_CORES):
        r = res.results[c]
        zqS = r["zqS"]                                      # [128, 4096]
        z_q[c * TOK : c * TOK + HT] = zqS[:D].T
        z_q[c * TOK + HT : (c + 1) * TOK] = zqS[D:].T
        idx_c = r["idxu"].T.reshape(-1).astype(np.int32)
        idx[c * TOK : (c + 1) * TOK] = idx_c
        lp = r["losspart"].astype(np.float64)               # [128, 8]
        S[2 * c] += lp[:D].sum()
        S[2 * c + 1] += lp[D:].sum()

        mx = r["maxes"]
        gap = (mx[:, :, 0] - mx[:, :, 1]).T.reshape(-1)
        Ac = A[c * TOK : (c + 1) * TOK]
        thr = 6.0 * np.spacing(Ac + np.float32(0.25))
        risky = np.nonzero(gap <= thr)[0]
        if risky.size:
            tg = c * TOK + risky
            Zr = zf[tg]
            Mr = Zr @ emb.T
            dr = (Ac[risky][:, None] + Bv[None, :]) - np.float32(2.0) * Mr
            k_new = dr.argmin(axis=1).astype(np.int32)
            chg = np.nonzero(k_new != idx[tg])[0]
            for i in chg:
                t = int(tg[i])
                zr = zf[t]
                q_old = z_q[t].copy()
                idx[t] = k_new[i]
                z_q[t] = zr + (emb[k_new[i]] - zr)
                b = t // T
                S[b] += float(
                    np.sum((z_q[t].astype(np.float64) - zr.astype(np.float64)) ** 2)
                    - np.sum((q_old.astype(np.float64) - zr.astype(np.float64)) ** 2)
                )

    cmean = (S / float(T * D)).astype(np.float32)
    tb = (np.float32(0.25) * cmean).astype(np.float32) + cmean
    loss = np.float32(tb.astype(np.float32).sum(dtype=np.float32))

    return (z_q.reshape(B, T, D), loss, idx.reshape(B, T).astype(np.int32))


# revision 15
# speedup vs baseline: 1.0625x; 1.0625x over previous
"""VQ codebook kernel for 8 Trainium2 NeuronCores (Bass/Tile).

For z [16, 4096, 64] f32 and emb [1024, 64] f32, computes (matching the
reference bit-for-bit on this machine):
  idx  = argmin_k ||z - e_k||^2   (fp32, XLA op order)    [16, 4096] int32
  z_q  = z + (emb[idx] - z)                               [16, 4096, 64] f32
  loss = sum_b (0.25*mean_b + mean_b), mean_b = mean((emb[idx]-z)^2)

Sharding: data-parallel; core c owns tokens [c*8192, (c+1)*8192) = batches
2c, 2c+1. Codebook replicated; loss partials combined on host.

Device pipeline per core (64 tiles of 128 tokens):
- PE: one fp32 matmul per 512-col half computes w = tree(2 z.e - B) straight
  into PSUM (zT carries an all-ones row 64; embT2 carries -B in row 64).
  The PE fp32 matmul is bit-identical to XLA-neuron's, so w reproduces the
  reference's distance ordering except at its coarse-grid rounding ties.
- DVE: max (top-8) + max_index per tile -> argmax of w = argmin of dist,
  first-occurrence tie-break; also diff = q - z and z_q = z + diff.
- GPSIMD: ap_gather ucode gathers q[d, t] = embT[d, idx[t]] (all 8 Q7 cores;
  batch-halves stacked across partitions 0-63 / 64-127).
- ACT: Square+accumulate for the loss partial sums.

Host: input shard/transpose prep, and a repair pass - tokens whose top-2 w
gap is within ~6 ulps of the reference's fl(fl(A+B)-2M) grid (~0.5%) get an
exact recompute, which also covers max_index tie semantics. A ulp-shift
argument shows A cannot otherwise affect the argmin, so it lives host-side
only.
"""

import sys

for _p in ("/opt/trn_rl_repo", "/root/.axon_site/_ro/trn_rl_repo"):
    if _p not in sys.path:
        sys.path.insert(0, _p)

import numpy as np

import concourse.bacc as bacc
import concourse.mybir as mybir
from concourse import library_config
from concourse.tile import TileContext
from concourse.bass_utils import run_bass_kernel_spmd

B, T, D, K = 16, 4096, 64, 1024
N_CORES = 8
TOK = (B * T) // N_CORES          # 8192 tokens per core
HT = TOK // 2                     # 4096 tokens per batch-half
N_TILES = TOK // 128              # 64 tiles
# Gather batch plan: (start tile within each half, #tiles). The last batch is
# split so the end-of-kernel gather tail is short.
BATCH_PLAN = [(0, 8), (8, 8), (16, 8), (24, 4), (28, 4)]
N_BATCH = len(BATCH_PLAN)

F32 = mybir.dt.float32
U16 = mybir.dt.uint16
I16 = mybir.dt.int16

_cached = {}


def _build_program():
    nc = bacc.Bacc(None, target_bir_lowering=False, debug=False)

    zT_d = nc.dram_tensor("zT", [D + 1, TOK], F32, kind="ExternalInput")
    embT2_d = nc.dram_tensor("embT2", [D + 1, K], F32, kind="ExternalInput")
    embT1_d = nc.dram_tensor("embT1", [128, K], F32, kind="ExternalInput")
    zS_d = nc.dram_tensor("zS", [128, HT], F32, kind="ExternalInput")

    zqS_d = nc.dram_tensor("zqS", [128, HT], F32, kind="ExternalOutput")
    idxu_d = nc.dram_tensor("idxu", [128, N_TILES], U16, kind="ExternalOutput")
    maxes_d = nc.dram_tensor("maxes", [128, N_TILES, 2], F32, kind="ExternalOutput")
    losspart_d = nc.dram_tensor("losspart", [128, N_BATCH], F32, kind="ExternalOutput")

    with TileContext(nc) as tc:
        with (
            tc.tile_pool(name="singles", bufs=1) as singles,
            tc.tile_pool(name="dfp", bufs=2) as dfp,
            tc.tile_pool(name="zqp", bufs=2) as zqp,
            tc.tile_pool(name="sqp", bufs=2) as sqp,
            tc.tile_pool(name="pswp", bufs=4, space="PSUM") as pswp,
        ):
            zT_sb = singles.tile([D + 1, TOK], F32)
            embT2_sb = singles.tile([D + 1, K], F32)
            embT1_sb = singles.tile([128, K, 1], F32)
            zS_sb = singles.tile([128, HT], F32)
            max8_all = singles.tile([128, N_TILES, 8], F32)
            idx8_all = singles.tile([128, N_TILES, 8], U16)
            idxw = singles.tile([128, HT // 16], I16)
            qS_all = singles.tile([128, HT, 1], F32)
            losspart_sb = singles.tile([128, N_BATCH], F32)

            nc.sync.dma_start(out=embT2_sb[:, 0:512], in_=embT2_d[:, 0:512])
            nc.sync.dma_start(out=embT2_sb[:, 512:1024], in_=embT2_d[:, 512:1024])
            for ch in range(16):
                cs = slice(ch * (TOK // 16), (ch + 1) * (TOK // 16))
                nc.sync.dma_start(out=zT_sb[:, cs], in_=zT_d[:, cs])
            nc.sync.dma_start(out=embT1_sb[:, :, 0], in_=embT1_d[:, :])
            for ch in range(4):
                cs = slice(ch * (HT // 4), (ch + 1) * (HT // 4))
                nc.sync.dma_start(out=zS_sb[:, cs], in_=zS_d[:, cs])

            nc.gpsimd.load_library(library_config.ap_gather)

            def _emit_elementwise(bb):
                ts_, nt_ = BATCH_PLAN[bb]
                gtok = slice(ts_ * 128, (ts_ + nt_) * 128)
                diff = dfp.tile([128, nt_ * 128], F32, tag="diff")
                nc.vector.tensor_tensor(
                    diff, qS_all[:, gtok, 0], zS_sb[:, gtok],
                    op=mybir.AluOpType.subtract,
                )
                zq = zqp.tile([128, nt_ * 128], F32, tag="zq")
                nc.vector.tensor_tensor(
                    zq, zS_sb[:, gtok], diff, op=mybir.AluOpType.add
                )
                sq = sqp.tile([128, nt_ * 128], F32, tag="sq")
                nc.scalar.activation(
                    sq, diff, mybir.ActivationFunctionType.Square,
                    accum_out=losspart_sb[:, bb : bb + 1],
                )
                nc.sync.dma_start(out=zqS_d[:, gtok], in_=zq)

            for b, (tstart, nt) in enumerate(BATCH_PLAN):
                bt = nt * 128                      # tokens per half this batch
                for half in range(2):
                    for j in range(nt):
                        g = half * (N_TILES // 2) + tstart + j
                        tok = slice(g * 128, (g + 1) * 128)
                        psw = pswp.tile([128, K], F32)
                        for h in range(2):
                            hs = slice(h * 512, (h + 1) * 512)
                            nc.tensor.matmul(
                                psw[:, hs], lhsT=zT_sb[:, tok], rhs=embT2_sb[:, hs],
                                start=True, stop=True,
                            )
                        nc.vector.max(out=max8_all[:, g, :], in_=psw)
                        nc.vector.max_index(idx8_all[:, g, :], max8_all[:, g, :], psw)

                # Wrapped idx lists: token i (within this batch's half) sits at
                # (partition i%16 + 64*half [+16*rep], col tstart*8 + i//16).
                # Source: idx8_all[p = i%128, tile = half*32 + tstart + i//128, 0].
                bcol = slice(tstart * 8, (tstart + nt) * 8)
                for half in range(2):
                    g0 = half * (N_TILES // 2) + tstart
                    pbase = 64 * half
                    for r in range(8):
                        nc.sync.dma_start(
                            out=idxw[pbase : pbase + 16,
                                     tstart * 8 + r : (tstart + nt) * 8 : 8],
                            in_=idx8_all[16 * r : 16 * (r + 1), g0 : g0 + nt, 0].bitcast(I16),
                        )
                    for rep in range(1, 4):
                        nc.sync.dma_start(
                            out=idxw[pbase + 16 * rep : pbase + 16 * (rep + 1), bcol],
                            in_=idxw[pbase : pbase + 16, bcol],
                        )

                nc.gpsimd.ap_gather(
                    qS_all[:, tstart * 128 : (tstart + nt) * 128, :],
                    embT1_sb,
                    idxw[:, bcol],
                    channels=128,
                    num_elems=K,
                    d=1,
                    num_idxs=bt,
                )

                # Software pipelining: emit batch b-1's elementwise phase here
                # so the DVE stream never waits on batch b's gather.
                if b > 0:
                    _emit_elementwise(b - 1)
            _emit_elementwise(N_BATCH - 1)

            nc.sync.dma_start(out=idxu_d[:, :], in_=idx8_all[:, :, 0])
            nc.sync.dma_start(out=maxes_d[:, :, :], in_=max8_all[:, :, 0:2])
            nc.sync.dma_start(out=losspart_d[:, :], in_=losspart_sb)

    nc.compile()
    return nc


def _get_program():
    if "nc" not in _cached:
        _cached["nc"] = _build_program()
    return _cached["nc"]


def kernel(z, emb):
    z = np.asarray(z, dtype=np.float32)
    emb = np.asarray(emb, dtype=np.float32)
    zf = z.reshape(-1, D)

    A = (zf * zf).sum(axis=1).astype(np.float32)
    Bv = (emb * emb).sum(axis=1).astype(np.float32)
    embT2 = np.empty((D + 1, K), np.float32)
    embT2[:D] = (emb * np.float32(2.0)).T
    embT2[D] = -Bv
    embT1 = np.empty((128, K), np.float32)
    embT1[:D] = emb.T
    embT1[D:] = emb.T

    in_maps = []
    for c in range(N_CORES):
        zc = zf[c * TOK : (c + 1) * TOK]
        z65 = np.empty((D + 1, TOK), np.float32)
        z65[:D] = zc.T
        z65[D] = 1.0
        zS = np.empty((128, HT), np.float32)
        zS[:D] = zc[:HT].T
        zS[D:] = zc[HT:].T
        in_maps.append({"zT": z65, "embT2": embT2, "embT1": embT1, "zS": zS})

    nc = _get_program()
    res = run_bass_kernel_spmd(
        nc, in_maps, core_ids=list(range(N_CORES)), **_cached.get("run_kwargs", {})
    )
    _cached["last_res"] = res

    z_q = np.empty((B * T, D), dtype=np.float32)
    idx = np.empty(B * T, dtype=np.int32)
    S = np.zeros(B, dtype=np.float64)

    for c in range(N_CORES):
        r = res.results[c]
        zqS = r["zqS"]                                      # [128, 4096]
        z_q[c * TOK : c * TOK + HT] = zqS[:D].T
        z_q[c * TOK + HT : (c + 1) * TOK] = zqS[D:].T
        idx_c = r["idxu"].T.reshape(-1).astype(np.int32)
        idx[c * TOK : (c + 1) * TOK] = idx_c
        lp = r["losspart"].astype(np.float64)               # [128, 4]
        S[2 * c] += lp[:D].sum()
        S[2 * c + 1] += lp[D:].sum()

        mx = r["maxes"]
        gap = (mx[:, :, 0] - mx[:, :, 1]).T.reshape(-1)
        Ac = A[c * TOK : (c + 1) * TOK]
        thr = 6.0 * np.spacing(Ac + np.float32(0.25))
        risky = np.nonzero(gap <= thr)[0]
        if risky.size:
            tg = c * TOK + risky
            Zr = zf[tg]
            Mr = Zr @ emb.T
            dr = (Ac[risky][:, None] + Bv[None, :]) - np.float32(2.0) * Mr
            k_new = dr.argmin(axis=1).astype(np.int32)
            chg = np.nonzero(k_new != idx[tg])[0]
            for i in chg:
                t = int(tg[i])
                zr = zf[t]
                q_old = z_q[t].copy()
                idx[t] = k_new[i]
                z_q[t] = zr + (emb[k_new[i]] - zr)
                b = t // T
                S[b] += float(
                    np.sum((z_q[t].astype(np.float64) - zr.astype(np.float64)) ** 2)
                    - np.sum((q_old.astype(np.float64) - zr.astype(np.float64)) ** 2)
                )

    cmean = (S / float(T * D)).astype(np.float32)
    tb = (np.float32(0.25) * cmean).astype(np.float32) + cmean
    loss = np.float32(tb.astype(np.float32).sum(dtype=np.float32))

    return (z_q.reshape(B, T, D), loss, idx.reshape(B, T).astype(np.int32))
